# revision 12
# baseline (speedup 1.0000x reference)
"""DLRM dot-interaction kernel for Trainium2 (Bass/Tile), 8-core data parallel.

Computes, for each sample b:
    combined = concat([dense[b], sparse[b]])          # [27, 128]
    C = combined @ combined.T                          # [27, 27] gram
    out[b] = concat([dense[b], triu_flat(C)])          # [506]

Device strategy (per core, S = 4096 samples):
  - Host pre-transposes inputs to X^T layout [D=128, S, 27] so the
    contraction dim D sits on SBUF partitions (avoids on-device transpose).
  - Per sample: one matmul lhsT=rhs=X_s^T [128,27] -> gram [27,27] in PSUM.
    Samples are packed 4-across the PE array via column tiling (PSUM
    partition offsets 0/32/64/96 -> tile_position (0,32g)) and 16-deep
    along each PSUM bank's free dim (16*27=432 of 512 fp32).
  - Each full bank (64 samples) is evicted to SBUF with one DVE copy.
  - One strided DMA per chunk writes grams to DRAM as [27, S, 27]
    (row-index major), which makes the host-side triu gather 27 cheap
    contiguous slice copies.
  - Dense passthrough (output cols 0:128) is pure data movement and is
    assembled on the host.
"""

import os
import sys

import numpy as np

for _p in (
    "/root/.axon_site",
    "/root/.axon_site/_ro/trn_rl_repo",
    "/opt/trn_rl_repo",
):
    if os.path.isdir(_p) and _p not in sys.path:
        sys.path.append(_p)

import concourse.bacc as bacc
import concourse.bass as bass
import concourse.mybir as mybir
import concourse.tile as tile

NF = 27  # combined features (1 dense + 26 sparse)
D = 128  # embedding dim
B = 32768  # batch
NCORES = 8
S = B // NCORES  # samples per core

F32 = mybir.dt.float32

# Upper-triangle (incl. diagonal) flattened offsets: row n starts at TOFF[n],
# length 27 - n. Matches np.triu_indices(27) row-major order.
TOFF = np.concatenate([[0], np.cumsum(NF - np.arange(NF))]).astype(np.int64)
NPAIRS = int(TOFF[NF])  # 378
DOUT = D + NPAIRS  # 506


def build_nc(s_per_core=S, jb=16, kb=8):
    """Build the per-core Bass program.

    s_per_core samples are processed in chunks of C = 4 * jb * kb samples:
    4 column groups x (kb PSUM banks x jb slots per bank).
    """
    j_tot = jb * kb
    c_sz = 4 * j_tot
    assert s_per_core % c_sz == 0, (s_per_core, c_sz)
    nchunks = s_per_core // c_sz

    # Bacc (not raw Bass): its compile() pass moves excess matmul sync-waits
    # onto event-semaphore ops — raw Bass emits >1 wait on LdWeights, which
    # walrus codegen rejects ("Too many sync wait commands").
    nc = bacc.Bacc("TRN2", target_bir_lowering=False, debug=False)
    # +5 pad columns: the stationary operand is widened from 27 to 32 columns
    # (M=32) so every PSUM partition of each 32-row column group is written
    # (rows 27-31 are garbage, never read back). The widened read of the last
    # sample in a chunk reaches 5 columns past its block, hence the pad.
    xt = nc.dram_tensor("xt", [D, s_per_core * NF + 5], F32, kind="ExternalInput")
    gram = nc.dram_tensor("gram", [NF, s_per_core, NF], F32, kind="ExternalOutput")

    bank_sz = 4 * jb  # samples per PSUM bank (4 column groups x jb slots)
    with tile.TileContext(nc) as tc:
        with (
            tc.tile_pool(name="xin", bufs=8) as xin_pool,
            tc.tile_pool(name="gbuf", bufs=2) as gbuf_pool,
            tc.tile_pool(name="ps", bufs=8, space="PSUM") as ps_pool,
        ):
            for c0 in range(nchunks):
                gbuf = gbuf_pool.tile([128, j_tot * NF], F32)
                for b in range(kb):
                    # Per-bank input load (bank_sz contiguous samples) keeps
                    # the DMA stream fine-grained so compute pipelines behind
                    # it instead of stalling on a whole-chunk transfer.
                    s_base = c0 * c_sz + b * bank_sz
                    xin = xin_pool.tile([D, bank_sz * NF + 5], F32)
                    nc.sync.dma_start(
                        out=xin[:],
                        in_=xt[:, s_base * NF : (s_base + bank_sz) * NF + 5],
                    )
                    ps = ps_pool.tile([128, jb * NF], F32)
                    # Tiny DVE write: absorbs the psum-slot-reuse dependency so
                    # the first matmul below carries <=2 sync waits (walrus
                    # limit on the LdWeights struct).
                    nc.vector.memset(ps[:, :1], 0.0)
                    for jbi in range(jb):
                        for g in range(4):
                            loc = (g * jb + jbi) * NF
                            nc.tensor.matmul(
                                ps[32 * g : 32 * g + 32, jbi * NF : (jbi + 1) * NF],
                                xin[:, loc : loc + 32],
                                xin[:, loc : loc + NF],
                                start=True,
                                stop=True,
                                tile_position=(0, 32 * g),
                            )
                    nc.vector.tensor_copy(
                        gbuf[:, b * jb * NF : (b + 1) * jb * NF], ps[:]
                    )
                # One DMA per column group g. Sample (c0, b, g, jbi) sits at
                # global index c0*c_sz + b*bank_sz + g*jb + jbi; for fixed g
                # that is dst dims (b, jbi) with strides (bank_sz, 1) in
                # samples, matching gbuf's contiguous (b, jbi, m) free dim.
                base = gram[:, c0 * c_sz : (c0 + 1) * c_sz, :].rearrange(
                    "p (b four j) m -> p b four j m", four=4, j=jb
                )
                for g in range(4):
                    nc.sync.dma_start(
                        out=base[:, :, g],
                        in_=gbuf[32 * g : 32 * g + NF, :],
                    )
    nc.finalize()  # runs Bacc.compile() (reg alloc, wait legalization)
    return nc


def host_pack_inputs(dense_features, sparse_features):
    """[B,128] + [B,26,128] -> X^T layout [128, B, 27] fp32."""
    bsz = dense_features.shape[0]
    xt = np.empty((D, bsz, NF), dtype=np.float32)
    xt[:, :, 0] = np.asarray(dense_features, dtype=np.float32).T
    xt[:, :, 1:] = np.asarray(sparse_features, dtype=np.float32).transpose(2, 0, 1)
    return xt


def host_core_input(xt, c, s_per_core=S):
    """Slice core c's shard and append the 5-column stationary pad."""
    flat = np.ascontiguousarray(
        xt[:, c * s_per_core : (c + 1) * s_per_core, :]
    ).reshape(D, s_per_core * NF)
    return np.concatenate(
        [flat, np.zeros((D, 5), dtype=np.float32)], axis=1
    )


def host_unpack_output(dense_features, gram_t):
    """dense [B,128] + gram_t [27, B, 27] -> [B, 506] (dense ++ triu)."""
    bsz = dense_features.shape[0]
    out = np.empty((bsz, DOUT), dtype=np.float32)
    out[:, :D] = dense_features
    for n in range(NF):
        lo = D + int(TOFF[n])
        out[:, lo : lo + NF - n] = gram_t[n, :, n:]
    return out


_NC_CACHE = {}


def _get_nc():
    key = (S,)
    if key not in _NC_CACHE:
        _NC_CACHE[key] = build_nc(S)
    return _NC_CACHE[key]


def kernel(dense_features, sparse_features):
    from concourse.bass_utils import run_bass_kernel_spmd

    dense_features = np.asarray(dense_features, dtype=np.float32)
    sparse_features = np.asarray(sparse_features, dtype=np.float32)
    xt = host_pack_inputs(dense_features, sparse_features)

    in_maps = [{"xt": host_core_input(xt, c)} for c in range(NCORES)]
    nc = _get_nc()
    res = run_bass_kernel_spmd(nc, in_maps, core_ids=list(range(NCORES)))
    gram_t = np.concatenate([r["gram"] for r in res.results], axis=1)  # [27, B, 27]
    return host_unpack_output(dense_features, gram_t)
